# revision 8
# baseline (speedup 1.0000x reference)
"""
Trainium2 kernel for nn_CanonicalLinear (dense_mlp).

Reference computation:
    heads[b, n, c] = x @ W[n].T + b[n]          (8 per-head linears)
    out[b, c]      = sum_n heads[b, n, c] * factor[n]

By linearity this collapses to a single linear layer:
    W_eff[c, d] = sum_n factor[n] * W[n, c, d]
    b_eff[c]    = sum_n factor[n] * b[n, c]
    out         = x @ W_eff.T + b_eff

which is 8x less matmul work than the naive per-head form.

Sharding over the 8 NeuronCores: 2-way data-parallel over the batch
(8192 -> 4096) x 4-way tensor-parallel over num_classes (2048 -> 512).
Core r handles batch half r//4 and class quarter r%4.  This minimizes
per-core HBM traffic (x 32MB + W 32MB + out 8MB = 72MB/core).

Per-core device kernel:
  1. DVE reduces W[n, c_slice, :] with factor weights -> W_eff slice.
  2. PE (tensor engine) transposes W_eff -> W_effT  [d, c]  (fp32 has no
     DMA transpose; transpose-mode matmuls with an identity are used).
  3. Per 128-row x tile: PE transposes x -> xT chunks, then accumulates
     out = xT.T @ W_effT over the 16 contraction chunks in PSUM.
     Matmuls run in float32r (FP22 reduced precision, 4x faster than
     true fp32 on the PE, rel err ~1e-4 for D=2048 dot products).
  4. DVE adds the broadcast bias during PSUM->SBUF eviction; DMA out.
"""

import numpy as np

P = 128
B, D, C, N = 8192, 2048, 2048, 8
DP, TP = 2, 4                      # data-parallel x tensor-parallel grid
BS, CS = B // DP, C // TP          # per-core batch rows / out cols
NCORES = DP * TP

_cached_nc = None


def _build(bs=BS, cs=CS, d=D, n_heads=N):
    import concourse.bass as bass
    import concourse.mybir as mybir
    import concourse.tile as tile
    from concourse import bacc
    from concourse.masks import make_identity

    FP32 = mybir.dt.float32
    F32R = mybir.dt.float32r
    MULT = mybir.AluOpType.mult
    ADD = mybir.AluOpType.add

    dk = d // P                    # contraction chunks
    cb = cs // P                   # c chunks per core
    nbt = bs // P                  # batch tiles per core

    nc = bacc.Bacc()
    xd = nc.dram_tensor("x", [bs, d], FP32, kind="ExternalInput")
    wd = nc.dram_tensor("w", [n_heads, cs, d], FP32, kind="ExternalInput")
    bd = nc.dram_tensor("b", [n_heads, cs], FP32, kind="ExternalInput")
    fd = nc.dram_tensor("f", [n_heads], FP32, kind="ExternalInput")
    od = nc.dram_tensor("out", [bs, cs], FP32, kind="ExternalOutput")

    with tile.TileContext(nc) as tc:
        with (
            tc.tile_pool(name="singles", bufs=1) as singles,
            tc.tile_pool(name="wload", bufs=3) as wload,
            tc.tile_pool(name="waccp", bufs=2) as waccp,
            tc.tile_pool(name="xload", bufs=4) as xload,
            tc.tile_pool(name="xtp", bufs=4) as xtp,
            tc.tile_pool(name="outp", bufs=3) as outp,
            tc.tile_pool(name="pst", bufs=2, space="PSUM") as pst,
            tc.tile_pool(name="psw", bufs=2, space="PSUM") as psw,
            tc.tile_pool(name="pso", bufs=2, space="PSUM") as pso,
        ):
            # --- constants ---------------------------------------------
            ident32 = singles.tile([P, P], FP32)
            make_identity(nc, ident32)
            ident_r = singles.tile([P, P], F32R)
            nc.vector.tensor_copy(ident_r, ident32)

            # factor broadcast to all 128 partitions: [P, N]
            f_ap = fd[:]
            f_rep = singles.tile([P, n_heads], FP32)
            nc.gpsimd.dma_start(
                f_rep,
                bass.AP(tensor=f_ap.tensor, offset=f_ap.offset,
                        ap=[[0, P]] + list(f_ap.ap)),
            )

            # bias broadcast to all partitions: [P, N, CS]
            b_ap = bd[:]
            b_rep = singles.tile([P, n_heads, cs], FP32)
            nc.gpsimd.dma_start(
                b_rep,
                bass.AP(tensor=b_ap.tensor, offset=b_ap.offset,
                        ap=[[0, P]] + list(b_ap.ap)),
            )

            # DVE copy absorbs the broadcast-DMA waits so the following
            # TensorScalar ops (single ISA wait slot) only ever wait on one
            # semaphore.
            f_use = singles.tile([P, n_heads], FP32)
            nc.vector.tensor_copy(f_use, f_rep)

            # Touch column: tiny DVE copies that absorb DMA-completion
            # semaphore waits, because TensorScalar ops have a single ISA
            # wait slot.
            touch = singles.tile([P, 48], FP32)
            nc.vector.tensor_copy(touch[:, 40:41], b_rep[:, 0, 0:1])

            # b_eff[c] = sum_n f[n] * b[n, c], replicated on partitions
            beff = singles.tile([P, cs], FP32)
            nc.vector.tensor_scalar(beff, b_rep[:, 0, :], f_use[:, 0:1],
                                    None, MULT)
            for n in range(1, n_heads):
                nc.vector.scalar_tensor_tensor(
                    beff, b_rep[:, n, :], f_use[:, n:n + 1], beff, MULT, ADD)

            # --- W phase: weighted reduce over heads, then transpose ----
            # weffT[dp, k, c] = W_eff[c, k*P + dp]
            weffT = singles.tile([P, dk, cs], F32R)
            for j in range(cb):
                wacc = waccp.tile([P, d], F32R)
                for n in range(n_heads):
                    wt = wload.tile([P, d], FP32)
                    nc.sync.dma_start(wt, wd[n, j * P:(j + 1) * P, :])
                    nc.vector.tensor_copy(touch[:, (8 * j + n) % 40:(8 * j + n) % 40 + 1],
                                          wt[:, 0:1])
                    if n == 0:
                        nc.vector.tensor_scalar(wacc, wt, f_use[:, 0:1],
                                                None, MULT)
                    else:
                        nc.vector.scalar_tensor_tensor(
                            wacc, wt, f_use[:, n:n + 1], wacc, MULT, ADD)
                # PE transpose wacc [c(=128), d] -> weffT chunks [d(=128), c]
                for g in range(dk // 4):
                    pw = psw.tile([P, 4, P], F32R)
                    for u in range(4):
                        k = 4 * g + u
                        nc.tensor.matmul(
                            pw[:, u, :],
                            wacc[:, k * P:(k + 1) * P],
                            ident_r,
                            is_transpose=True,
                        )
                    nc.any.tensor_copy(
                        weffT[:, 4 * g:4 * g + 4, j * P:(j + 1) * P], pw)

            # --- main loop over 128-row x tiles -------------------------
            for i in range(nbt):
                xtile = xload.tile([P, d], F32R)
                nc.sync.dma_start(xtile, xd[i * P:(i + 1) * P, :].bitcast(F32R))
                xt = xtp.tile([P, dk, P], F32R)
                for g in range(dk // 4):
                    pt = pst.tile([P, 4, P], F32R)
                    for u in range(4):
                        k = 4 * g + u
                        nc.tensor.matmul(
                            pt[:, u, :],
                            xtile[:, k * P:(k + 1) * P],
                            ident_r,
                            is_transpose=True,
                        )
                    nc.any.tensor_copy(xt[:, 4 * g:4 * g + 4, :], pt)

                po = pso.tile([P, cs], FP32)
                for k in range(dk):
                    nc.tensor.matmul(
                        po,
                        xt[:, k, :],
                        weffT[:, k, :],
                        start=(k == 0),
                        stop=(k == dk - 1),
                    )
                osb = outp.tile([P, cs], FP32)
                nc.vector.tensor_add(osb, po, beff)
                nc.sync.dma_start(od[i * P:(i + 1) * P, :], osb)

    nc.finalize()
    return nc


def _get_nc():
    global _cached_nc
    if _cached_nc is None:
        _cached_nc = _build()
    return _cached_nc


def _shard_inputs(x, W, b, factor):
    in_maps = []
    for r in range(NCORES):
        p, q = divmod(r, TP)
        in_maps.append({
            "x": np.ascontiguousarray(x[p * BS:(p + 1) * BS]),
            "w": np.ascontiguousarray(W[:, q * CS:(q + 1) * CS, :]),
            "b": np.ascontiguousarray(b[:, q * CS:(q + 1) * CS]),
            "f": np.ascontiguousarray(factor),
        })
    return in_maps


def kernel(x, W, b, factor, _trace=False):
    from concourse.bass_utils import run_bass_kernel_spmd

    x = np.asarray(x, dtype=np.float32)
    W = np.asarray(W, dtype=np.float32)
    b = np.asarray(b, dtype=np.float32)
    factor = np.asarray(factor, dtype=np.float32)

    nc = _get_nc()
    in_maps = _shard_inputs(x, W, b, factor)
    res = run_bass_kernel_spmd(nc, in_maps, list(range(NCORES)),
                               trace=_trace)

    out = np.empty((B, C), dtype=np.float32)
    for r in range(NCORES):
        p, q = divmod(r, TP)
        out[p * BS:(p + 1) * BS, q * CS:(q + 1) * CS] = res.results[r]["out"]
    if _trace:
        return out, res
    return out
